# revision 13
# baseline (speedup 1.0000x reference)
"""Multi-head attention (N=4, L=2048, C=1024, H=16, D=64) on 8 TRN2 NeuronCores.

Sharding: core c -> batch n = c//2, head-group g = c%2 (8 heads each).
Each core computes its 8 heads' attention + the partial output projection
for batch n; the host sums the two partials per batch and adds the
constant bias term (b_out + b_v @ W_out).

Device-side layout (per core):
  xT   [C=1024, L=2048]  bf16 (x[n].T, host-transposed/cast)
  wqk  [C, 1024]         bf16 (W_in cols: 8 heads' q dims then k dims)
  wv   [C, 512]          bf16 (W_in cols: 8 heads' v dims)
  wo   [512, F=1024]     bf16 (W_out rows for the 8 heads)
  bqk  [128, 8]          f32  (q/k bias, partition-major per j-tile)
  y    [L, F]            f32  output partial

Pipeline: qT/kT = W^T @ xT (j on partitions), V = xT^T @ Wv (l on partitions),
scoresT[k, q] per head via row-tiled head pairs (K=64), exp on ACT
(scale=1/8 fused, fp32 PSUM -> bf16 SBUF), AV^T + row-sums on PE
(col-tiled pairs + M=1 ones matmuls), normalize via DMA-broadcast
reciprocal, final projection y = outT^T @ W_out.
"""

import os
import sys
from contextlib import ExitStack

import numpy as np

sys.path.insert(0, "/opt/trn_rl_repo")

import ml_dtypes

import concourse.bass as bass
import concourse.tile as tile
from concourse import bacc, mybir
from concourse.bass_utils import run_bass_kernel_spmd

BF16 = mybir.dt.bfloat16
F32 = mybir.dt.float32
FT = mybir.ActivationFunctionType

N, L, C, H, D = 4, 2048, 1024, 16, 64
QKV = H * D  # 1024
F = 1024  # output feature dim
HG = 8  # heads per core
NCORES = 8
SCALE = float(D) ** -0.5  # 0.125

# Globals for test harness introspection
TRACE = False
LAST_RESULTS = None


def _build_program() -> bass.Bass:
    nc = bacc.Bacc()

    xT_d = nc.declare_dram_parameter("xT", [C, L], BF16, isOutput=False)
    wqk_d = nc.declare_dram_parameter("wqk", [C, 1024], BF16, isOutput=False)
    wv_d = nc.declare_dram_parameter("wv", [C, 512], BF16, isOutput=False)
    wo_d = nc.declare_dram_parameter("wo", [512, F], BF16, isOutput=False)
    bqk_d = nc.declare_dram_parameter("bqk", [128, 8], F32, isOutput=False)
    y_d = nc.declare_dram_parameter("y", [L, F], F32, isOutput=True)

    CT = C // 128  # 8 c-tiles
    LT = L // 128  # 16 l-tiles
    JQ = L // 512  # 4 q-chunks
    KT = L // 128  # 16 k-tiles
    NP = HG // 2  # 4 head pairs

    with tile.TileContext(nc) as tc, ExitStack() as ctx:
        const_pool = ctx.enter_context(tc.tile_pool(name="const", bufs=1))
        qk_pool = ctx.enter_context(tc.tile_pool(name="qkT", bufs=1))
        v_pool = ctx.enter_context(tc.tile_pool(name="V", bufs=1))
        outT_pool = ctx.enter_context(tc.tile_pool(name="outT", bufs=1))
        exp_pool = ctx.enter_context(tc.tile_pool(name="expT", bufs=2))
        rbc_pool = ctx.enter_context(tc.tile_pool(name="rbc", bufs=2))
        r_pool = ctx.enter_context(tc.tile_pool(name="r", bufs=2))
        y_pool = ctx.enter_context(tc.tile_pool(name="y", bufs=2))
        dram_pool = ctx.enter_context(tc.tile_pool(name="scr", bufs=2, space="DRAM"))
        ps_s = ctx.enter_context(tc.tile_pool(name="ps_s", bufs=2, space="PSUM"))
        ps_avA = ctx.enter_context(tc.tile_pool(name="ps_avA", bufs=2, space="PSUM"))
        ps_avB = ctx.enter_context(tc.tile_pool(name="ps_avB", bufs=2, space="PSUM"))

        ones = const_pool.tile([128, 1], BF16)
        nc.vector.memset(ones[:], 1.0)
        bqk_sb = const_pool.tile([128, 8], F32)
        nc.sync.dma_start(bqk_sb[:], bqk_d[:])

        # qT/kT: [128, jt(8), jl(4), 512] ; jt 0-3 q dims, 4-7 k dims.
        qkT_sb = qk_pool.tile([128, 8, 4, 512], BF16)
        # V: [128, lt(16), 512]
        V_sb = v_pool.tile([128, LT, 512], BF16)
        # outT: [128, pair(4), L] (partitions = 2 heads x 64 dims)
        outT_sb = outT_pool.tile([128, NP, L], BF16)

        # ---- Phase 1: projections (xT/wqk/wv live only here) ----
        with tc.tile_pool(name="proj", bufs=1) as proj_pool:
            xT_sb = proj_pool.tile([128, CT, L], BF16)
            nc.sync.dma_start(xT_sb[:], xT_d.rearrange("(t p) l -> p t l", p=128))
            wqk_sb = proj_pool.tile([128, CT, 1024], BF16)
            nc.sync.dma_start(wqk_sb[:], wqk_d.rearrange("(t p) j -> p t j", p=128))
            wv_sb = proj_pool.tile([128, CT, 512], BF16)
            nc.sync.dma_start(wv_sb[:], wv_d.rearrange("(t p) j -> p t j", p=128))

            # qkT[j, l] = sum_c wqk[c, j] * xT[c, l]
            for jt in range(8):
                for lh in range(2):  # halves of L (1024 cols each)
                    ps = ps_s.tile([128, 2, 512], F32, tag="s")
                    for ct in range(CT):
                        for lc in range(2):
                            nc.tensor.matmul(
                                ps[:, lc],
                                lhsT=wqk_sb[:, ct, jt * 128 : (jt + 1) * 128],
                                rhs=xT_sb[:, ct, lh * 1024 + lc * 512 : lh * 1024 + (lc + 1) * 512],
                                start=(ct == 0),
                                stop=(ct == CT - 1),
                            )
                    # + bias, cast to bf16
                    nc.vector.tensor_scalar_add(
                        qkT_sb[:, jt, 2 * lh : 2 * lh + 2, :],
                        ps[:],
                        bqk_sb[:, jt : jt + 1],
                    )

            # V[l, j] = sum_c xT[c, l] * wv[c, j]   (no bias: folded into host)
            for lt in range(LT):
                psv = ps_avB.tile([128, 512], F32, tag="avB")
                for ct in range(CT):
                    nc.tensor.matmul(
                        psv[:],
                        lhsT=xT_sb[:, ct, lt * 128 : (lt + 1) * 128],
                        rhs=wv_sb[:, ct, :],
                        start=(ct == 0),
                        stop=(ct == CT - 1),
                    )
                nc.vector.tensor_copy(V_sb[:, lt, :], psv[:])

        # ---- Phase 2: attention per head pair ----
        for p in range(NP):
            hA, hB = 2 * p, 2 * p + 1
            for jq in range(JQ):
                expT = exp_pool.tile([128, KT, 2, 512], BF16)
                qA = qkT_sb[0:64, p, jq, :]
                qB = qkT_sb[64:128, p, jq, :]
                for kt in range(KT):
                    S = ps_s.tile([128, 2, 512], F32, tag="s")
                    jl, off = kt // 4, (kt % 4) * 128
                    # scoresT[k, q] = sum_d kT[d, k] qT[d, q]  (row-tiled pair)
                    nc.tensor.matmul(
                        S[:, 0],
                        lhsT=qkT_sb[0:64, 4 + p, jl, off : off + 128],
                        rhs=qA,
                        start=True,
                        stop=True,
                    )
                    nc.tensor.matmul(
                        S[:, 1],
                        lhsT=qkT_sb[64:128, 4 + p, jl, off : off + 128],
                        rhs=qB,
                        start=True,
                        stop=True,
                    )
                    nc.scalar.activation(expT[:, kt], S[:], FT.Exp, scale=SCALE)

                # AV^T: head A -> avA rows 0:64 (col groups 0-1), head B ->
                # avB rows 64:128 (col groups 2-3). Separate PSUM banks so the
                # two accumulation groups can interleave (start=True clears
                # has_written bank-wide).
                avA = ps_avA.tile([97, 512], F32, tag="avA")
                avB = ps_avB.tile([128, 512], F32, tag="avB")
                for kt in range(KT):
                    st, sp = kt == 0, kt == KT - 1
                    nc.tensor.matmul(
                        avA[0:64],
                        lhsT=V_sb[:, kt, hA * 64 : hA * 64 + 64],
                        rhs=expT[:, kt, 0],
                        start=st,
                        stop=sp,
                    )
                    nc.tensor.matmul(
                        avB[64:128],
                        lhsT=V_sb[:, kt, hB * 64 : hB * 64 + 64],
                        rhs=expT[:, kt, 1],
                        start=st,
                        stop=sp,
                    )
                # Row sums via ones-matmul, placed in free 32-aligned rows of
                # the same banks (groups start only after the AV groups stop).
                for kt in range(KT):
                    st, sp = kt == 0, kt == KT - 1
                    nc.tensor.matmul(
                        avA[96:97],
                        lhsT=ones[:],
                        rhs=expT[:, kt, 0],
                        start=st,
                        stop=sp,
                        tile_position=(0, 96),
                    )
                    nc.tensor.matmul(
                        avB[0:1], lhsT=ones[:], rhs=expT[:, kt, 1], start=st, stop=sp
                    )

                # reciprocal of row sums; broadcast across partitions via DRAM
                r_sb = r_pool.tile([97, 2, 512], F32)
                nc.vector.reciprocal(r_sb[96:97, 0], avA[96:97])
                nc.vector.reciprocal(r_sb[0:1, 1], avB[0:1])
                scr = dram_pool.tile([2, 512], F32)
                nc.sync.dma_start(scr[0:1], r_sb[96:97, 0])
                nc.sync.dma_start(scr[1:2], r_sb[0:1, 1])
                rbc = rbc_pool.tile([128, 512], F32)
                nc.sync.dma_start(rbc[0:64], scr[0:1].to_broadcast([64, 512]))
                nc.sync.dma_start(rbc[64:128], scr[1:2].to_broadcast([64, 512]))
                nc.vector.tensor_tensor(
                    outT_sb[0:64, p, jq * 512 : (jq + 1) * 512],
                    avA[0:64],
                    rbc[0:64],
                    mybir.AluOpType.mult,
                )
                nc.vector.tensor_tensor(
                    outT_sb[64:128, p, jq * 512 : (jq + 1) * 512],
                    avB[64:128],
                    rbc[64:128],
                    mybir.AluOpType.mult,
                )

        # ---- Phase 3: y[l, f] = sum_d outT[d, l] wo[d, f] ----
        with tc.tile_pool(name="wo", bufs=1) as wo_pool:
            # wo: [512, F] -> [128, 4, F] (d-tile p holds rows p*128..+128)
            wo_sb = wo_pool.tile([128, 4, F], BF16)
            nc.sync.dma_start(wo_sb[:], wo_d.rearrange("(t p) f -> p t f", p=128))
            for lt in range(LT):
                psy = ps_s.tile([128, 2, 512], F32, tag="s")
                for fc in range(2):
                    for p in range(NP):
                        nc.tensor.matmul(
                            psy[:, fc],
                            lhsT=outT_sb[:, p, lt * 128 : (lt + 1) * 128],
                            rhs=wo_sb[:, p, fc * 512 : (fc + 1) * 512],
                            start=(p == 0),
                            stop=(p == NP - 1),
                        )
                y_sb = y_pool.tile([128, 1024], F32)
                nc.vector.tensor_copy(y_sb[:, 0:512], psy[:, 0])
                nc.vector.tensor_copy(y_sb[:, 512:1024], psy[:, 1])
                nc.sync.dma_start(y_d[lt * 128 : (lt + 1) * 128, :], y_sb[:])

    nc.finalize()
    return nc


_NC_CACHE = None


def _get_program():
    global _NC_CACHE
    if _NC_CACHE is None:
        _NC_CACHE = _build_program()
    return _NC_CACHE


def _make_in_maps(x, W_in, b_in, W_out):
    bf = ml_dtypes.bfloat16
    in_maps = []
    for c in range(NCORES):
        n, g = c // 2, c % 2
        h0 = g * HG  # first global head
        j0 = h0 * D  # 512*g
        xT = np.ascontiguousarray(x[n].T).astype(bf)  # [C, L]
        wqk = np.concatenate(
            [W_in[:, j0 : j0 + 512], W_in[:, QKV + j0 : QKV + j0 + 512]], axis=1
        ).astype(bf)
        wv = np.ascontiguousarray(W_in[:, 2 * QKV + j0 : 2 * QKV + j0 + 512]).astype(bf)
        wo = np.ascontiguousarray(W_out[j0 : j0 + 512, :]).astype(bf)
        bqk = (
            np.concatenate([b_in[j0 : j0 + 512], b_in[QKV + j0 : QKV + j0 + 512]])
            .astype(np.float32)
            .reshape(8, 128)
            .T.copy()
        )
        in_maps.append({"xT": xT, "wqk": wqk, "wv": wv, "wo": wo, "bqk": bqk})
    return in_maps


def kernel(x, W_in, b_in, W_out, b_out):
    global LAST_RESULTS
    x = np.asarray(x, dtype=np.float32)
    W_in = np.asarray(W_in, dtype=np.float32)
    b_in = np.asarray(b_in, dtype=np.float32)
    W_out = np.asarray(W_out, dtype=np.float32)
    b_out = np.asarray(b_out, dtype=np.float32)

    nc = _get_program()
    in_maps = _make_in_maps(x, W_in, b_in, W_out)
    res = run_bass_kernel_spmd(
        nc, in_maps, list(range(NCORES)), trace=TRACE
    )
    LAST_RESULTS = res

    # host bias: b_out + b_v @ W_out  (b_v enters linearly through the
    # softmax-normalized value average: A@(V+b_v) = A@V + b_v)
    host_bias = (
        b_out.astype(np.float64) + b_in[2 * QKV :].astype(np.float64) @ W_out.astype(np.float64)
    ).astype(np.float32)

    out = np.empty((N, L, F), dtype=np.float32)
    for n in range(N):
        y0 = np.asarray(res.results[2 * n]["y"], dtype=np.float32)
        y1 = np.asarray(res.results[2 * n + 1]["y"], dtype=np.float32)
        out[n] = y0 + y1 + host_bias
    return out
